# revision 33
# baseline (speedup 1.0000x reference)
"""Transformer-XL relative-position multi-head attention on 8 Trainium2 cores.

Sharding: tensor-parallel over heads (16 heads -> 2 per core), data kept
full-batch on every core.  Each core computes its 2 heads' attention and a
partial output projection (Wo row-shard); the host sums the 8 partials.

Per-core dataflow (all layouts "transposed": d or j on partitions):
  qT/kT/rkT = W.T @ xT projections (kt-outer loops: each weight tile is
             loaded into the PE once and streamed over all column chunks).
  BD term:   RELT[i,u] = (q+r_r_bias)[i] . rk[u]  (per i-tile, PSUM)
             -> bf16 -> DRAM bounce -> read back with a stride-(2047) flat
             access pattern (== Transformer-XL rel_shift restricted to the
             causal region) fused with an XBAR transpose -> BD^T[j,i] tiles.
  scores^T:  AC^T = k . (q+r_w_bias) matmul into PSUM, ACT exp(scale=1/8)
             -> probs^T bf16, DVE multiplies by the bounced exp(s*BD)^T
             (bf16 mult at 2x DVE rate), affine_select zeroes the causal
             boundary (which also kills rel-shift garbage).
  AV:        probs^T j-partitioned matmul with [v | 1] (ones column gives the
             softmax denominator), jt-outer with two interleaved PSUM banks,
             normalize, bf16 out-projection -> partials summed on host.
"""

import sys

for _p in ("/opt/trn_rl_repo", "/root/.axon_site/_ro/trn_rl_repo"):
    if _p not in sys.path:
        sys.path.insert(0, _p)

import numpy as np
import ml_dtypes

import concourse.bass as bass
import concourse.bacc as bacc
import concourse.mybir as mybir
import concourse.tile as tile
from concourse.bass import MemorySpace
from concourse.masks import make_identity

# ---------------------------------------------------------------- constants
TOT_LEN, MEM_LEN, BSZ = 1024, 1024, 4
SEG_LEN = TOT_LEN + MEM_LEN          # 2048
D_EMBED, N_HEAD, D_HEAD = 1024, 16, 64
N_CORES = 8
NH_LOC = N_HEAD // N_CORES           # 2 heads per core
DH = NH_LOC * D_HEAD                 # 128
SCALE = 1.0 / (D_HEAD ** 0.5)        # 1/8

FP = mybir.dt.float32
F32R = mybir.dt.float32r
BF = mybir.dt.bfloat16

I_TILES = TOT_LEN // 128             # 8
J_TILES = SEG_LEN // 128             # 16
K_TILES = D_EMBED // 128             # 8
NCH = SEG_LEN // 512                 # 4 column chunks of cat
# packed causal-windowed probsT column offsets per j-tile
_PT_OFF = []
_acc = 0
for _jt in range(J_TILES):
    _PT_OFF.append(_acc)
    _acc += TOT_LEN - max(0, _jt * 128 - MEM_LEN)
PT_COLS = _acc                       # 12800

_PROGRAM = None


def _build_program(dbg=False, reps=1):
    """Build the SPMD per-core Bass program (identical on all 8 cores)."""
    nc = bacc.Bacc("TRN2", target_bir_lowering=False, debug=False)

    # DRAM I/O ------------------------------------------------------------
    catT = nc.dram_tensor("catT", [D_EMBED, BSZ, SEG_LEN], BF, kind="ExternalInput")
    rT = nc.dram_tensor("rT", [D_EMBED, SEG_LEN], BF, kind="ExternalInput")
    wq = nc.dram_tensor("wq", [D_EMBED, DH], BF, kind="ExternalInput")
    wk = nc.dram_tensor("wk", [D_EMBED, DH], BF, kind="ExternalInput")
    wv = nc.dram_tensor("wv", [D_EMBED, DH], BF, kind="ExternalInput")
    wr = nc.dram_tensor("wr", [D_EMBED, DH], BF, kind="ExternalInput")
    wo = nc.dram_tensor("wo", [DH, D_EMBED], BF, kind="ExternalInput")
    bias_w = nc.dram_tensor("bias_w", [DH, 1], FP, kind="ExternalInput")
    bias_r = nc.dram_tensor("bias_r", [DH, 1], FP, kind="ExternalInput")
    out = nc.dram_tensor("out", [BSZ, TOT_LEN, D_EMBED], BF, kind="ExternalOutput")

    dbg_t = {}
    if dbg:
        dbg_t["qwT"] = nc.dram_tensor("dbg_qwT", [DH, TOT_LEN], BF, kind="ExternalOutput")
        dbg_t["qrT"] = nc.dram_tensor("dbg_qrT", [DH, TOT_LEN], BF, kind="ExternalOutput")
        dbg_t["kT"] = nc.dram_tensor("dbg_kT", [DH, SEG_LEN], BF, kind="ExternalOutput")
        dbg_t["rkT"] = nc.dram_tensor("dbg_rkT", [DH, SEG_LEN], BF, kind="ExternalOutput")
        dbg_t["v"] = nc.dram_tensor("dbg_v", [128, J_TILES, NH_LOC, D_HEAD + 1], BF, kind="ExternalOutput")
        dbg_t["bdt"] = nc.dram_tensor("dbg_bdt", [128, J_TILES, TOT_LEN], BF, kind="ExternalOutput")
        dbg_t["probsT"] = nc.dram_tensor("dbg_probsT", [128, J_TILES, TOT_LEN], BF, kind="ExternalOutput")
        dbg_t["avt"] = nc.dram_tensor("dbg_avt", [DH, TOT_LEN], BF, kind="ExternalOutput")

    with tile.TileContext(nc) as tc:
        _emit(nc, tc, catT, rT, wq, wk, wv, wr, wo, bias_w, bias_r, out, dbg_t, reps)

    nc.compile()
    return nc


def _emit(nc, tc, catT, rT, wq, wk, wv, wr, wo, bias_w, bias_r, out, dbg_t={}, reps=1):
    from contextlib import ExitStack

    ctx = ExitStack()
    with ctx:
        consts = ctx.enter_context(tc.tile_pool(name="consts", bufs=1))
        ctp = ctx.enter_context(tc.tile_pool(name="ctp", bufs=1))
        stream = ctx.enter_context(tc.tile_pool(name="stream", bufs=3))
        projp = ctx.enter_context(tc.tile_pool(name="projp", bufs=2))
        reltp = ctx.enter_context(tc.tile_pool(name="reltp", bufs=4))
        bdtp = ctx.enter_context(tc.tile_pool(name="bdtp", bufs=8))
        probp = ctx.enter_context(tc.tile_pool(name="probp", bufs=2))
        avtp = ctx.enter_context(tc.tile_pool(name="avtp", bufs=2))
        outp = ctx.enter_context(tc.tile_pool(name="outp", bufs=2))
        smallp = ctx.enter_context(tc.tile_pool(name="smallp", bufs=2))
        psB = ctx.enter_context(tc.tile_pool(name="psB", bufs=6, space=MemorySpace.PSUM))
        psAV = ctx.enter_context(tc.tile_pool(name="psAV", bufs=2, space=MemorySpace.PSUM))
        dramp = ctx.enter_context(tc.tile_pool(name="dramp", bufs=8, space="DRAM"))

        def psum_tile(name):
            return psB.tile([128, 512], FP, tag="ps", name=name)

        # ---------------- constants into SBUF ----------------
        # weight layout for lhsT: (128 part, K_TILES, DH) with w_sb[p, kt, :] = W[kt*128+p, :]
        def load_w(w_dram, name, eng):
            t = consts.tile([128, K_TILES, DH], BF, name=name, tag=name)
            eng.dma_start(t[:], w_dram.rearrange("(kt p) d -> p kt d", p=128))
            return t

        wq_sb = load_w(wq, "wq_sb", nc.scalar)
        wk_sb = load_w(wk, "wk_sb", nc.scalar)
        wv_sb = load_w(wv, "wv_sb", nc.scalar)
        wr_sb = load_w(wr, "wr_sb", nc.scalar)
        ident = consts.tile([128, 128], FP, name="ident", tag="ident")
        make_identity(nc, ident[:])
        wo_sb = consts.tile([DH, D_EMBED], BF)
        nc.scalar.dma_start(wo_sb[:], wo[:])
        zt = consts.tile([128, 7, 128], BF, name="zt", tag="zt")
        nc.vector.memset(zt[:], 0.0)
        bw_sb = consts.tile([DH, 1], FP)
        br_sb = consts.tile([DH, 1], FP)
        nc.scalar.dma_start(bw_sb[:], bias_w[:])
        nc.scalar.dma_start(br_sb[:], bias_r[:])

        # ---------------- rkT projection: (DH part, SEG_LEN) ----------------
        rkT_sb = consts.tile([DH, SEG_LEN], BF)
        for _rep in range(reps):
          for c0 in range(0, SEG_LEN, 512):
              ps = psum_tile("rk_ps")
              rt_t = stream.tile([128, K_TILES, 512], BF, tag="instream", name="rt_t")
              nc.scalar.dma_start(rt_t[:], rT.rearrange("(kt p) s -> p kt s", p=128)[:, :, c0:c0 + 512])
              for kt in range(K_TILES):
                  nc.tensor.matmul(ps[:], wr_sb[:, kt, :],
                                   rt_t[:, kt, :],
                                   start=(kt == 0), stop=(kt == K_TILES - 1))
              nc.vector.tensor_copy(rkT_sb[:, c0:c0 + 512], ps[:])

          # ---------------- per batch ----------------
          for b in range(BSZ):
              qwT = projp.tile([DH, TOT_LEN], BF, tag="qwT")
              qrT = projp.tile([DH, TOT_LEN], BF, tag="qrT")
              kT = projp.tile([DH, SEG_LEN], BF, tag="kT")
              # v: j-partitioned, per (j-tile, head): (128, jt, n, 65); col 64 = ones
              v_sb = projp.tile([128, J_TILES, NH_LOC, D_HEAD + 1], BF, tag="v")
              nc.vector.memset(v_sb[:, :, :, 64], 1.0)

              # whole-batch cat slab resident; per-kt DMAs release deps early
              ct = ctp.tile([128, K_TILES, SEG_LEN], BF, tag="ct", name="ct")
              nc.scalar.dma_start(
                  ct[:],
                  catT.rearrange("(kt p) b s -> p kt b s", p=128)[:, :, b, :])

              # k-projection: kt-outer, 4 chunk banks
              kpss = [psum_tile("kps") for _ in range(NCH)]
              for kt in range(K_TILES):
                  for c in range(NCH):
                      nc.tensor.matmul(kpss[c][:], wk_sb[:, kt, :],
                                       ct[:, kt, c * 512:(c + 1) * 512],
                                       start=(kt == 0), stop=(kt == K_TILES - 1))
              for c in range(NCH):
                  nc.vector.tensor_copy(kT[:, c * 512:(c + 1) * 512], kpss[c][:])

              # q-projection (cat cols TOT..SEG = chunks 2,3): kt-outer
              qpss = [psum_tile("qps") for _ in range(2)]
              for kt in range(K_TILES):
                  for ci, c in enumerate((2, 3)):
                      nc.tensor.matmul(qpss[ci][:], wq_sb[:, kt, :],
                                       ct[:, kt, c * 512:(c + 1) * 512],
                                       start=(kt == 0), stop=(kt == K_TILES - 1))
              for ci in range(2):
                  i0c = ci * 512
                  nc.vector.tensor_scalar_add(qwT[:, i0c:i0c + 512], qpss[ci][:], bw_sb[:])
                  nc.vector.tensor_scalar_add(qrT[:, i0c:i0c + 512], qpss[ci][:], br_sb[:])

              # v-projection: kt-outer, then PE-transpose to j-partitioned
              vpss = [psum_tile("vps") for _ in range(NCH)]
              for kt in range(K_TILES):
                  for c in range(NCH):
                      nc.tensor.matmul(vpss[c][:], wv_sb[:, kt, :],
                                       ct[:, kt, c * 512:(c + 1) * 512],
                                       start=(kt == 0), stop=(kt == K_TILES - 1))
              for c in range(NCH):
                  vtc = stream.tile([128, 512], FP, tag="vtc", name="vtc")
                  nc.vector.tensor_copy(vtc[:], vpss[c][:])
                  vtp = psum_tile("vtp")
                  for jj in range(4):
                      jt = c * 4 + jj
                      nc.tensor.transpose(vtp[:, jj * 128:(jj + 1) * 128],
                                          vtc[:, jj * 128:(jj + 1) * 128], ident[:])
                      nc.vector.tensor_copy(v_sb[:, jt, :, 0:64],
                                            vtp[:, jj * 128:(jj + 1) * 128].rearrange("p (n d) -> p n d", n=NH_LOC))

              if dbg_t and b == 0:
                  nc.scalar.dma_start(dbg_t["qwT"][:], qwT[:])
                  nc.scalar.dma_start(dbg_t["qrT"][:], qrT[:])
                  nc.scalar.dma_start(dbg_t["kT"][:], kT[:])
                  nc.scalar.dma_start(dbg_t["rkT"][:], rkT_sb[:])
                  nc.scalar.dma_start(dbg_t["v"][:], v_sb[:])

              # AV^T accumulator for both heads, bf16: (128 = n*64+d, TOT_LEN)
              avt_sb = avtp.tile([DH, TOT_LEN], BF, tag="avt")

              bounces = []
              for n in range(NH_LOC):
                  p_lo, p_hi = n * 64, (n + 1) * 64
                  # ---- 2a: RELT per i-tile -> bf16 -> DRAM bounce ----
                  bounce = dramp.tile([TOT_LEN, SEG_LEN], BF, name=f"bounce{n}")
                  bounces.append(bounce)
                  bz = bass.AP(
                      tensor=bounce[:].tensor,
                      offset=bounce[:].offset,
                      ap=[[SEG_LEN, 128], [128 * SEG_LEN, 7], [1, 128]],
                  )
                  nc.gpsimd.dma_start(bz, zt[:])
                  for it in range(I_TILES):
                      i0 = it * 128
                      u_lo = (TOT_LEN - 128) - i0          # 896 - i0
                      relt_sb = reltp.tile([128, SEG_LEN], BF, tag="relt", name="relt_sb")
                      for ci, c0 in enumerate(range(u_lo, SEG_LEN, 512)):
                          cw = min(512, SEG_LEN - c0)
                          rps = psum_tile("rps")
                          nc.tensor.matmul(rps[:, 0:cw],
                                           qrT[p_lo:p_hi, i0:i0 + 128],
                                           rkT_sb[p_lo:p_hi, c0:c0 + cw],
                                           start=True, stop=True)
                          nc.scalar.activation(relt_sb[:, c0:c0 + cw], rps[:, 0:cw],
                              mybir.ActivationFunctionType.Exp, scale=SCALE)
                      nc.sync.dma_start(bounce[i0:i0 + 128, u_lo:SEG_LEN],
                                          relt_sb[:, u_lo:SEG_LEN])

              # ---- 2b: interleave both heads at j-tile granularity ----
              probsTs = [probp.tile([128, PT_COLS], BF, tag="probsT",
                                    name=f"probsT{n}") for n in range(NH_LOC)]
              for jt in range(J_TILES):
                  j0 = jt * 128
                  i_start = max(0, j0 - MEM_LEN)
                  iw = TOT_LEN - i_start
                  for n in range(NH_LOC):
                      p_lo, p_hi = n * 64, (n + 1) * 64
                      bflat = bounces[n][:]
                      probsT = probsTs[n]
                      # shifted + transposed read of the bounce buffer:
                      # BD[i, j] = RELT[i, j + 1023 - i] == flat[i*2047 + j + 1023]
                      bdt = bdtp.tile([128, TOT_LEN], BF, tag="bdt", name="bdt")
                      src = bass.AP(
                          tensor=bflat.tensor,
                          offset=bflat.offset + i_start * (SEG_LEN - 1) + j0 + (TOT_LEN - 1),
                          ap=[[SEG_LEN - 1, iw], [1, 128]],
                      )
                      nc.sync.dma_start(bdt[:, 0:iw], src, transpose=True)
                      if dbg_t and b == 0 and n == 0:
                          nc.scalar.dma_start(dbg_t["bdt"][:, jt, 0:iw], bdt[:, 0:iw])

                      for c0 in range(i_start, TOT_LEN, 512):
                          cw = min(512, TOT_LEN - c0)
                          acps = psum_tile("acps")
                          bsl = bdt[:, c0 - i_start:c0 - i_start + cw]
                          psl = probsT[:, _PT_OFF[jt] + c0 - i_start:_PT_OFF[jt] + c0 - i_start + cw]
                          nc.tensor.matmul(acps[:, 0:cw],
                                           kT[p_lo:p_hi, j0:j0 + 128],
                                           qwT[p_lo:p_hi, c0:c0 + cw],
                                           start=True, stop=True)
                          nc.scalar.activation(psl, acps[:, 0:cw],
                              mybir.ActivationFunctionType.Exp, scale=SCALE)
                          # probs = exp(s*AC) * exp(s*BD): bf16 mult at 2x DVE rate
                          nc.vector.tensor_mul(psl, psl, bsl)
                      if jt == J_TILES - 1:
                          # last j-tile's wrap-reads land in rows with u_lo=0
                          # (real data, not pre-zeroed): mask explicitly
                          nc.gpsimd.affine_select(
                              out=probsT[:, _PT_OFF[jt]:_PT_OFF[jt] + 128],
                              in_=probsT[:, _PT_OFF[jt]:_PT_OFF[jt] + 128],
                              compare_op=mybir.AluOpType.is_ge,
                              fill=0.0, base=0, channel_multiplier=-1,
                              pattern=[[1, 128]],
                          )

              if dbg_t and b == 0:
                  for jt in range(J_TILES):
                      i_s = max(0, jt * 128 - MEM_LEN)
                      nc.scalar.dma_start(dbg_t["probsT"][:, jt, i_s:TOT_LEN],
                                          probsTs[0][:, _PT_OFF[jt]:_PT_OFF[jt] + TOT_LEN - i_s])

              # ---- 2c: AV + normalize (jt-outer, 2 interleaved banks) ----
              for n in range(NH_LOC):
                  p_lo, p_hi = n * 64, (n + 1) * 64
                  probsT = probsTs[n]
                  avpss = [psAV.tile([128, 512], FP, tag="avps", name="avps") for _ in range(2)]
                  # last jt contributing to chunk c0: any jt with i_s < c0+512
                  last_jt = [max(jt for jt in range(J_TILES)
                                 if max(0, jt * 128 - MEM_LEN) < c0 + 512)
                             for c0 in (0, 512)]
                  for jt in range(J_TILES):
                      i_s = max(0, jt * 128 - MEM_LEN)
                      for ci, c0 in enumerate((0, 512)):
                          if i_s >= c0 + 512:
                              continue
                          lo = max(c0, i_s)
                          nc.tensor.matmul(avpss[ci][0:D_HEAD + 1, lo - c0:512],
                                           v_sb[:, jt, n, :],
                                           probsT[:, _PT_OFF[jt] + lo - i_s:_PT_OFF[jt] + c0 + 512 - i_s],
                                           start=(jt == 0), stop=(jt == last_jt[ci]))
                  for ci, c0 in enumerate((0, 512)):
                      avps = avpss[ci]
                      recip = smallp.tile([1, 512], FP, tag="recip", name="recip")
                      rbc = smallp.tile([64, 512], FP, tag="rbc", name="rbc")
                      nc.vector.reciprocal(recip[:], avps[64:65, :])
                      nc.gpsimd.partition_broadcast(rbc[:], recip[:])
                      nc.vector.tensor_mul(avt_sb[p_lo:p_hi, c0:c0 + 512],
                                           avps[0:64, :], rbc[:])

              if dbg_t and b == 0:
                  nc.scalar.dma_start(dbg_t["avt"][:], avt_sb[:])

              # ---- 3: partial output projection for batch b ----
              # two i-tiles per ot tile -> one out DMA per 256 rows
              for it2 in range(I_TILES // 2):
                  ot = outp.tile([128, 2, 1024], BF, tag="ot")
                  for ii in range(2):
                      i0 = (2 * it2 + ii) * 128
                      for ec in range(2):
                          ops = psum_tile("ops")
                          nc.tensor.matmul(ops[:],
                                           avt_sb[:, i0:i0 + 128],
                                           wo_sb[:, ec * 512:(ec + 1) * 512],
                                           start=True, stop=True)
                          nc.vector.tensor_copy(ot[:, ii, ec * 512:(ec + 1) * 512], ops[:])
                  nc.scalar.dma_start(
                      out[b, it2 * 256:(it2 + 1) * 256, :].rearrange(
                          "(ii p) e -> p ii e", p=128),
                      ot[:])


def _get_program():
    global _PROGRAM
    if _PROGRAM is None:
        _PROGRAM = _build_program()
    return _PROGRAM


def _prep_inputs(w, r, r_w_bias, r_r_bias, attn_mask, mems, Wqkv, Wr, Wo):
    """Host-side sharding: returns list of 8 per-core input dicts."""
    bf16 = ml_dtypes.bfloat16
    cat = np.concatenate([mems, w], axis=0)               # (S, b, E)
    catT = np.ascontiguousarray(cat.transpose(2, 1, 0)).astype(bf16)  # (E, b, S)
    rT = np.ascontiguousarray(r.T).astype(bf16)           # (E, S)

    in_maps = []
    for core in range(N_CORES):
        n0 = core * NH_LOC
        cs, ce = n0 * D_HEAD, (n0 + NH_LOC) * D_HEAD
        in_maps.append({
            "catT": catT,
            "rT": rT,
            "wq": np.ascontiguousarray(Wqkv[:, cs:ce]).astype(bf16),
            "wk": np.ascontiguousarray(Wqkv[:, D_EMBED + cs:D_EMBED + ce]).astype(bf16),
            "wv": np.ascontiguousarray(Wqkv[:, 2 * D_EMBED + cs:2 * D_EMBED + ce]).astype(bf16),
            "wr": np.ascontiguousarray(Wr[:, cs:ce]).astype(bf16),
            "wo": np.ascontiguousarray(Wo[cs:ce, :]).astype(bf16),
            "bias_w": np.ascontiguousarray(r_w_bias[n0:n0 + NH_LOC].reshape(DH, 1)),
            "bias_r": np.ascontiguousarray(r_r_bias[n0:n0 + NH_LOC].reshape(DH, 1)),
        })
    return in_maps


def kernel(w, r, r_w_bias, r_r_bias, attn_mask, mems, Wqkv, Wr, Wo):
    from concourse.bass_utils import run_bass_kernel_spmd

    nc = _get_program()
    in_maps = _prep_inputs(w, r, r_w_bias, r_r_bias, attn_mask, mems, Wqkv, Wr, Wo)
    res = run_bass_kernel_spmd(nc, in_maps, list(range(N_CORES)))
    # out per core: (b, i, e) bf16 partial; sum over cores (head groups)
    total = np.zeros((BSZ, TOT_LEN, D_EMBED), np.float32)
    for core in range(N_CORES):
        total += res.results[core]["out"].astype(np.float32)
    return np.ascontiguousarray(total.transpose(1, 0, 2))  # (i, b, e)


# revision 34
# speedup vs baseline: 1.0438x; 1.0438x over previous
"""Transformer-XL relative-position multi-head attention on 8 Trainium2 cores.

Sharding: tensor-parallel over heads (16 heads -> 2 per core), data kept
full-batch on every core.  Each core computes its 2 heads' attention and a
partial output projection (Wo row-shard); the host sums the 8 partials.

Per-core dataflow (all layouts "transposed": d or j on partitions):
  qT/kT/rkT = W.T @ xT projections (kt-outer loops: each weight tile is
             loaded into the PE once and streamed over all column chunks).
  BD term:   RELT[i,u] = (q+r_r_bias)[i] . rk[u]  (per i-tile, PSUM)
             -> bf16 -> DRAM bounce -> read back with a stride-(2047) flat
             access pattern (== Transformer-XL rel_shift restricted to the
             causal region) fused with an XBAR transpose -> BD^T[j,i] tiles.
  scores^T:  AC^T = k . (q+r_w_bias) matmul into PSUM, ACT exp(scale=1/8)
             -> probs^T bf16, DVE multiplies by the bounced exp(s*BD)^T
             (bf16 mult at 2x DVE rate), affine_select zeroes the causal
             boundary (which also kills rel-shift garbage).
  AV:        probs^T j-partitioned matmul with [v | 1] (ones column gives the
             softmax denominator), jt-outer with two interleaved PSUM banks,
             normalize, bf16 out-projection -> partials summed on host.
"""

import sys

for _p in ("/opt/trn_rl_repo", "/root/.axon_site/_ro/trn_rl_repo"):
    if _p not in sys.path:
        sys.path.insert(0, _p)

import numpy as np
import ml_dtypes

import concourse.bass as bass
import concourse.bacc as bacc
import concourse.mybir as mybir
import concourse.tile as tile
from concourse.bass import MemorySpace
from concourse.masks import make_identity

# ---------------------------------------------------------------- constants
TOT_LEN, MEM_LEN, BSZ = 1024, 1024, 4
SEG_LEN = TOT_LEN + MEM_LEN          # 2048
D_EMBED, N_HEAD, D_HEAD = 1024, 16, 64
N_CORES = 8
NH_LOC = N_HEAD // N_CORES           # 2 heads per core
DH = NH_LOC * D_HEAD                 # 128
SCALE = 1.0 / (D_HEAD ** 0.5)        # 1/8

FP = mybir.dt.float32
F32R = mybir.dt.float32r
BF = mybir.dt.bfloat16

I_TILES = TOT_LEN // 128             # 8
J_TILES = SEG_LEN // 128             # 16
K_TILES = D_EMBED // 128             # 8
NCH = SEG_LEN // 512                 # 4 column chunks of cat
# packed causal-windowed probsT column offsets per j-tile
_PT_OFF = []
_acc = 0
for _jt in range(J_TILES):
    _PT_OFF.append(_acc)
    _acc += TOT_LEN - max(0, _jt * 128 - MEM_LEN)
PT_COLS = _acc                       # 12800

_PROGRAM = None


def _build_program(dbg=False, reps=1):
    """Build the SPMD per-core Bass program (identical on all 8 cores)."""
    nc = bacc.Bacc("TRN2", target_bir_lowering=False, debug=False)

    # DRAM I/O ------------------------------------------------------------
    catT = nc.dram_tensor("catT", [D_EMBED, BSZ, SEG_LEN], BF, kind="ExternalInput")
    rT = nc.dram_tensor("rT", [D_EMBED, SEG_LEN], BF, kind="ExternalInput")
    wq = nc.dram_tensor("wq", [D_EMBED, DH], BF, kind="ExternalInput")
    wk = nc.dram_tensor("wk", [D_EMBED, DH], BF, kind="ExternalInput")
    wv = nc.dram_tensor("wv", [D_EMBED, DH], BF, kind="ExternalInput")
    wr = nc.dram_tensor("wr", [D_EMBED, DH], BF, kind="ExternalInput")
    wo = nc.dram_tensor("wo", [DH, D_EMBED], BF, kind="ExternalInput")
    bias_w = nc.dram_tensor("bias_w", [DH, 1], FP, kind="ExternalInput")
    bias_r = nc.dram_tensor("bias_r", [DH, 1], FP, kind="ExternalInput")
    out = nc.dram_tensor("out", [BSZ, TOT_LEN, D_EMBED], BF, kind="ExternalOutput")

    dbg_t = {}
    if dbg:
        dbg_t["qwT"] = nc.dram_tensor("dbg_qwT", [DH, TOT_LEN], BF, kind="ExternalOutput")
        dbg_t["qrT"] = nc.dram_tensor("dbg_qrT", [DH, TOT_LEN], BF, kind="ExternalOutput")
        dbg_t["kT"] = nc.dram_tensor("dbg_kT", [DH, SEG_LEN], BF, kind="ExternalOutput")
        dbg_t["rkT"] = nc.dram_tensor("dbg_rkT", [DH, SEG_LEN], BF, kind="ExternalOutput")
        dbg_t["v"] = nc.dram_tensor("dbg_v", [128, J_TILES, NH_LOC, D_HEAD + 1], BF, kind="ExternalOutput")
        dbg_t["bdt"] = nc.dram_tensor("dbg_bdt", [128, J_TILES, TOT_LEN], BF, kind="ExternalOutput")
        dbg_t["probsT"] = nc.dram_tensor("dbg_probsT", [128, J_TILES, TOT_LEN], BF, kind="ExternalOutput")
        dbg_t["avt"] = nc.dram_tensor("dbg_avt", [DH, TOT_LEN], BF, kind="ExternalOutput")

    with tile.TileContext(nc) as tc:
        _emit(nc, tc, catT, rT, wq, wk, wv, wr, wo, bias_w, bias_r, out, dbg_t, reps)

    nc.compile()
    return nc


def _emit(nc, tc, catT, rT, wq, wk, wv, wr, wo, bias_w, bias_r, out, dbg_t={}, reps=1):
    from contextlib import ExitStack

    ctx = ExitStack()
    with ctx:
        consts = ctx.enter_context(tc.tile_pool(name="consts", bufs=1))
        ctp = ctx.enter_context(tc.tile_pool(name="ctp", bufs=1))
        stream = ctx.enter_context(tc.tile_pool(name="stream", bufs=3))
        projp = ctx.enter_context(tc.tile_pool(name="projp", bufs=2))
        reltp = ctx.enter_context(tc.tile_pool(name="reltp", bufs=4))
        bdtp = ctx.enter_context(tc.tile_pool(name="bdtp", bufs=8))
        probp = ctx.enter_context(tc.tile_pool(name="probp", bufs=2))
        avtp = ctx.enter_context(tc.tile_pool(name="avtp", bufs=2))
        outp = ctx.enter_context(tc.tile_pool(name="outp", bufs=4))
        smallp = ctx.enter_context(tc.tile_pool(name="smallp", bufs=2))
        psB = ctx.enter_context(tc.tile_pool(name="psB", bufs=6, space=MemorySpace.PSUM))
        psAV = ctx.enter_context(tc.tile_pool(name="psAV", bufs=2, space=MemorySpace.PSUM))
        dramp = ctx.enter_context(tc.tile_pool(name="dramp", bufs=8, space="DRAM"))

        def psum_tile(name):
            return psB.tile([128, 512], FP, tag="ps", name=name)

        # ---------------- constants into SBUF ----------------
        # weight layout for lhsT: (128 part, K_TILES, DH) with w_sb[p, kt, :] = W[kt*128+p, :]
        def load_w(w_dram, name, eng):
            t = consts.tile([128, K_TILES, DH], BF, name=name, tag=name)
            eng.dma_start(t[:], w_dram.rearrange("(kt p) d -> p kt d", p=128))
            return t

        wq_sb = load_w(wq, "wq_sb", nc.scalar)
        wk_sb = load_w(wk, "wk_sb", nc.scalar)
        wv_sb = load_w(wv, "wv_sb", nc.scalar)
        wr_sb = load_w(wr, "wr_sb", nc.scalar)
        ident = consts.tile([128, 128], FP, name="ident", tag="ident")
        make_identity(nc, ident[:])
        wo_sb = consts.tile([DH, D_EMBED], BF)
        nc.scalar.dma_start(wo_sb[:], wo[:])
        zt = consts.tile([128, 7, 128], BF, name="zt", tag="zt")
        nc.vector.memset(zt[:], 0.0)
        bw_sb = consts.tile([DH, 1], FP)
        br_sb = consts.tile([DH, 1], FP)
        nc.scalar.dma_start(bw_sb[:], bias_w[:])
        nc.scalar.dma_start(br_sb[:], bias_r[:])

        # ---------------- rkT projection: (DH part, SEG_LEN) ----------------
        rkT_sb = consts.tile([DH, SEG_LEN], BF)
        for _rep in range(reps):
          for c0 in range(0, SEG_LEN, 512):
              ps = psum_tile("rk_ps")
              rt_t = stream.tile([128, K_TILES, 512], BF, tag="instream", name="rt_t")
              nc.scalar.dma_start(rt_t[:], rT.rearrange("(kt p) s -> p kt s", p=128)[:, :, c0:c0 + 512])
              for kt in range(K_TILES):
                  nc.tensor.matmul(ps[:], wr_sb[:, kt, :],
                                   rt_t[:, kt, :],
                                   start=(kt == 0), stop=(kt == K_TILES - 1))
              nc.vector.tensor_copy(rkT_sb[:, c0:c0 + 512], ps[:])

          # ---------------- per batch ----------------
          for b in range(BSZ):
              qwT = projp.tile([DH, TOT_LEN], BF, tag="qwT")
              qrT = projp.tile([DH, TOT_LEN], BF, tag="qrT")
              kT = projp.tile([DH, SEG_LEN], BF, tag="kT")
              # v: j-partitioned, per (j-tile, head): (128, jt, n, 65); col 64 = ones
              v_sb = projp.tile([128, J_TILES, NH_LOC, D_HEAD + 1], BF, tag="v")
              nc.vector.memset(v_sb[:, :, :, 64], 1.0)

              # whole-batch cat slab resident; per-kt DMAs release deps early
              ct = ctp.tile([128, K_TILES, SEG_LEN], BF, tag="ct", name="ct")
              for kt in range(K_TILES):
                  nc.scalar.dma_start(
                      ct[:, kt, :],
                      catT.rearrange("(kt p) b s -> p kt b s", p=128)[:, kt, b, :])

              # k-projection: kt-outer, 4 chunk banks
              kpss = [psum_tile("kps") for _ in range(NCH)]
              for kt in range(K_TILES):
                  for c in range(NCH):
                      nc.tensor.matmul(kpss[c][:], wk_sb[:, kt, :],
                                       ct[:, kt, c * 512:(c + 1) * 512],
                                       start=(kt == 0), stop=(kt == K_TILES - 1))
              for c in range(NCH):
                  nc.vector.tensor_copy(kT[:, c * 512:(c + 1) * 512], kpss[c][:])

              # q-projection (cat cols TOT..SEG = chunks 2,3): kt-outer
              qpss = [psum_tile("qps") for _ in range(2)]
              for kt in range(K_TILES):
                  for ci, c in enumerate((2, 3)):
                      nc.tensor.matmul(qpss[ci][:], wq_sb[:, kt, :],
                                       ct[:, kt, c * 512:(c + 1) * 512],
                                       start=(kt == 0), stop=(kt == K_TILES - 1))
              for ci in range(2):
                  i0c = ci * 512
                  nc.vector.tensor_scalar_add(qwT[:, i0c:i0c + 512], qpss[ci][:], bw_sb[:])
                  nc.vector.tensor_scalar_add(qrT[:, i0c:i0c + 512], qpss[ci][:], br_sb[:])

              # v-projection: kt-outer, then PE-transpose to j-partitioned
              vpss = [psum_tile("vps") for _ in range(NCH)]
              for kt in range(K_TILES):
                  for c in range(NCH):
                      nc.tensor.matmul(vpss[c][:], wv_sb[:, kt, :],
                                       ct[:, kt, c * 512:(c + 1) * 512],
                                       start=(kt == 0), stop=(kt == K_TILES - 1))
              for c in range(NCH):
                  vtc = stream.tile([128, 512], FP, tag="vtc", name="vtc")
                  nc.vector.tensor_copy(vtc[:], vpss[c][:])
                  vtp = psum_tile("vtp")
                  for jj in range(4):
                      jt = c * 4 + jj
                      nc.tensor.transpose(vtp[:, jj * 128:(jj + 1) * 128],
                                          vtc[:, jj * 128:(jj + 1) * 128], ident[:])
                      nc.vector.tensor_copy(v_sb[:, jt, :, 0:64],
                                            vtp[:, jj * 128:(jj + 1) * 128].rearrange("p (n d) -> p n d", n=NH_LOC))

              if dbg_t and b == 0:
                  nc.scalar.dma_start(dbg_t["qwT"][:], qwT[:])
                  nc.scalar.dma_start(dbg_t["qrT"][:], qrT[:])
                  nc.scalar.dma_start(dbg_t["kT"][:], kT[:])
                  nc.scalar.dma_start(dbg_t["rkT"][:], rkT_sb[:])
                  nc.scalar.dma_start(dbg_t["v"][:], v_sb[:])

              # AV^T accumulator for both heads, bf16: (128 = n*64+d, TOT_LEN)
              avt_sb = avtp.tile([DH, TOT_LEN], BF, tag="avt")

              bounces = []
              for n in range(NH_LOC):
                  p_lo, p_hi = n * 64, (n + 1) * 64
                  # ---- 2a: RELT per i-tile -> bf16 -> DRAM bounce ----
                  bounce = dramp.tile([TOT_LEN, SEG_LEN], BF, name=f"bounce{n}")
                  bounces.append(bounce)
                  bz = bass.AP(
                      tensor=bounce[:].tensor,
                      offset=bounce[:].offset,
                      ap=[[SEG_LEN, 128], [128 * SEG_LEN, 7], [1, 128]],
                  )
                  nc.gpsimd.dma_start(bz, zt[:])
                  for it in range(I_TILES):
                      i0 = it * 128
                      u_lo = (TOT_LEN - 128) - i0          # 896 - i0
                      relt_sb = reltp.tile([128, SEG_LEN], BF, tag="relt", name="relt_sb")
                      for ci, c0 in enumerate(range(u_lo, SEG_LEN, 512)):
                          cw = min(512, SEG_LEN - c0)
                          rps = psum_tile("rps")
                          nc.tensor.matmul(rps[:, 0:cw],
                                           qrT[p_lo:p_hi, i0:i0 + 128],
                                           rkT_sb[p_lo:p_hi, c0:c0 + cw],
                                           start=True, stop=True)
                          nc.scalar.activation(relt_sb[:, c0:c0 + cw], rps[:, 0:cw],
                              mybir.ActivationFunctionType.Exp, scale=SCALE)
                      nc.sync.dma_start(bounce[i0:i0 + 128, u_lo:SEG_LEN],
                                          relt_sb[:, u_lo:SEG_LEN])

              # ---- 2b: interleave both heads at j-tile granularity ----
              probsTs = [probp.tile([128, PT_COLS], BF, tag="probsT",
                                    name=f"probsT{n}") for n in range(NH_LOC)]
              for jt in range(J_TILES):
                  j0 = jt * 128
                  i_start = max(0, j0 - MEM_LEN)
                  iw = TOT_LEN - i_start
                  for n in range(NH_LOC):
                      p_lo, p_hi = n * 64, (n + 1) * 64
                      bflat = bounces[n][:]
                      probsT = probsTs[n]
                      # shifted + transposed read of the bounce buffer:
                      # BD[i, j] = RELT[i, j + 1023 - i] == flat[i*2047 + j + 1023]
                      bdt = bdtp.tile([128, TOT_LEN], BF, tag="bdt", name="bdt")
                      src = bass.AP(
                          tensor=bflat.tensor,
                          offset=bflat.offset + i_start * (SEG_LEN - 1) + j0 + (TOT_LEN - 1),
                          ap=[[SEG_LEN - 1, iw], [1, 128]],
                      )
                      nc.sync.dma_start(bdt[:, 0:iw], src, transpose=True)
                      if dbg_t and b == 0 and n == 0:
                          nc.scalar.dma_start(dbg_t["bdt"][:, jt, 0:iw], bdt[:, 0:iw])

                      for c0 in range(i_start, TOT_LEN, 512):
                          cw = min(512, TOT_LEN - c0)
                          acps = psum_tile("acps")
                          bsl = bdt[:, c0 - i_start:c0 - i_start + cw]
                          psl = probsT[:, _PT_OFF[jt] + c0 - i_start:_PT_OFF[jt] + c0 - i_start + cw]
                          nc.tensor.matmul(acps[:, 0:cw],
                                           kT[p_lo:p_hi, j0:j0 + 128],
                                           qwT[p_lo:p_hi, c0:c0 + cw],
                                           start=True, stop=True)
                          nc.scalar.activation(psl, acps[:, 0:cw],
                              mybir.ActivationFunctionType.Exp, scale=SCALE)
                          # probs = exp(s*AC) * exp(s*BD): bf16 mult at 2x DVE rate
                          nc.vector.tensor_mul(psl, psl, bsl)
                      if jt == J_TILES - 1:
                          # last j-tile's wrap-reads land in rows with u_lo=0
                          # (real data, not pre-zeroed): mask explicitly
                          nc.gpsimd.affine_select(
                              out=probsT[:, _PT_OFF[jt]:_PT_OFF[jt] + 128],
                              in_=probsT[:, _PT_OFF[jt]:_PT_OFF[jt] + 128],
                              compare_op=mybir.AluOpType.is_ge,
                              fill=0.0, base=0, channel_multiplier=-1,
                              pattern=[[1, 128]],
                          )

              if dbg_t and b == 0:
                  for jt in range(J_TILES):
                      i_s = max(0, jt * 128 - MEM_LEN)
                      nc.scalar.dma_start(dbg_t["probsT"][:, jt, i_s:TOT_LEN],
                                          probsTs[0][:, _PT_OFF[jt]:_PT_OFF[jt] + TOT_LEN - i_s])

              # ---- 2c: AV + normalize (jt-outer, 2 interleaved banks) ----
              for n in range(NH_LOC):
                  p_lo, p_hi = n * 64, (n + 1) * 64
                  probsT = probsTs[n]
                  avpss = [psAV.tile([128, 512], FP, tag="avps", name="avps") for _ in range(2)]
                  # last jt contributing to chunk c0: any jt with i_s < c0+512
                  last_jt = [max(jt for jt in range(J_TILES)
                                 if max(0, jt * 128 - MEM_LEN) < c0 + 512)
                             for c0 in (0, 512)]
                  for jt in range(J_TILES):
                      i_s = max(0, jt * 128 - MEM_LEN)
                      for ci, c0 in enumerate((0, 512)):
                          if i_s >= c0 + 512:
                              continue
                          lo = max(c0, i_s)
                          nc.tensor.matmul(avpss[ci][0:D_HEAD + 1, lo - c0:512],
                                           v_sb[:, jt, n, :],
                                           probsT[:, _PT_OFF[jt] + lo - i_s:_PT_OFF[jt] + c0 + 512 - i_s],
                                           start=(jt == 0), stop=(jt == last_jt[ci]))
                  for ci, c0 in enumerate((0, 512)):
                      avps = avpss[ci]
                      recip = smallp.tile([1, 512], FP, tag="recip", name="recip")
                      rbc = smallp.tile([64, 512], FP, tag="rbc", name="rbc")
                      nc.vector.reciprocal(recip[:], avps[64:65, :])
                      nc.gpsimd.partition_broadcast(rbc[:], recip[:])
                      nc.vector.tensor_mul(avt_sb[p_lo:p_hi, c0:c0 + 512],
                                           avps[0:64, :], rbc[:])

              if dbg_t and b == 0:
                  nc.scalar.dma_start(dbg_t["avt"][:], avt_sb[:])

              # ---- 3: partial output projection for batch b ----
              for it in range(I_TILES):
                  i0 = it * 128
                  ot = outp.tile([128, 1024], BF, tag="ot")
                  for ec in range(2):
                      ops = psum_tile("ops")
                      nc.tensor.matmul(ops[:],
                                       avt_sb[:, i0:i0 + 128],
                                       wo_sb[:, ec * 512:(ec + 1) * 512],
                                       start=True, stop=True)
                      nc.vector.tensor_copy(ot[:, ec * 512:(ec + 1) * 512], ops[:])
                  nc.scalar.dma_start(out[b, i0:i0 + 128, :], ot[:])


def _get_program():
    global _PROGRAM
    if _PROGRAM is None:
        _PROGRAM = _build_program()
    return _PROGRAM


def _prep_inputs(w, r, r_w_bias, r_r_bias, attn_mask, mems, Wqkv, Wr, Wo):
    """Host-side sharding: returns list of 8 per-core input dicts."""
    bf16 = ml_dtypes.bfloat16
    cat = np.concatenate([mems, w], axis=0)               # (S, b, E)
    catT = np.ascontiguousarray(cat.transpose(2, 1, 0)).astype(bf16)  # (E, b, S)
    rT = np.ascontiguousarray(r.T).astype(bf16)           # (E, S)

    in_maps = []
    for core in range(N_CORES):
        n0 = core * NH_LOC
        cs, ce = n0 * D_HEAD, (n0 + NH_LOC) * D_HEAD
        in_maps.append({
            "catT": catT,
            "rT": rT,
            "wq": np.ascontiguousarray(Wqkv[:, cs:ce]).astype(bf16),
            "wk": np.ascontiguousarray(Wqkv[:, D_EMBED + cs:D_EMBED + ce]).astype(bf16),
            "wv": np.ascontiguousarray(Wqkv[:, 2 * D_EMBED + cs:2 * D_EMBED + ce]).astype(bf16),
            "wr": np.ascontiguousarray(Wr[:, cs:ce]).astype(bf16),
            "wo": np.ascontiguousarray(Wo[cs:ce, :]).astype(bf16),
            "bias_w": np.ascontiguousarray(r_w_bias[n0:n0 + NH_LOC].reshape(DH, 1)),
            "bias_r": np.ascontiguousarray(r_r_bias[n0:n0 + NH_LOC].reshape(DH, 1)),
        })
    return in_maps


def kernel(w, r, r_w_bias, r_r_bias, attn_mask, mems, Wqkv, Wr, Wo):
    from concourse.bass_utils import run_bass_kernel_spmd

    nc = _get_program()
    in_maps = _prep_inputs(w, r, r_w_bias, r_r_bias, attn_mask, mems, Wqkv, Wr, Wo)
    res = run_bass_kernel_spmd(nc, in_maps, list(range(N_CORES)))
    # out per core: (b, i, e) bf16 partial; sum over cores (head groups)
    total = np.zeros((BSZ, TOT_LEN, D_EMBED), np.float32)
    for core in range(N_CORES):
        total += res.results[core]["out"].astype(np.float32)
    return np.ascontiguousarray(total.transpose(1, 0, 2))  # (i, b, e)
